# revision 43
# baseline (speedup 1.0000x reference)
"""Trainium2 Bass kernel for nn_Attention_65180423685043 (sparse_attention).

Sharding: 8 cores = 4 batches x 2 sequence-halves. Each core computes the
spatial-reduction/kv branch for its batch (duplicated within the pair) and
attention + the up branch only for its 2048-token half. No collectives.

Host prep ships layout-transformed inputs (no arithmetic on x beyond dtype
cast): channel-major padded image ximg, transposed xqT, and diagonal-expanded
depthwise conv weights. Two compiled variants exist because the up-conv tap
row offset (16*half) is a compile-time AP constant.
"""
import numpy as np
import ml_dtypes

import concourse.bass as bass
import concourse.mybir as mybir
import concourse.tile as tile
from concourse import bacc
from concourse.bass_utils import run_bass_kernel_spmd
from concourse.masks import make_identity

F32 = mybir.dt.float32
BF16 = mybir.dt.bfloat16
F8 = mybir.dt.float8e4
AF = mybir.ActivationFunctionType
ALU = mybir.AluOpType

B, N, C, HEADS, SR = 4, 4096, 256, 4, 2
HD = C // HEADS
H = W = 64
Hs = Ws = 32
M = Hs * Ws          # kv tokens
NT = N // 2          # tokens per core
SCALE = HD ** -0.5
EPS = 1e-6

BF_INPUTS = {"ximg", "xqT", "srdgi", "updgi", "qwt", "kwt", "vwt",
             "pwt", "pwtu", "vb", "pb", "sel", "wcsn"}


def build_nc(half):
    nc = bacc.Bacc("TRN2", target_bir_lowering=False, debug=False, num_devices=8)
    d = {}

    def din(name, shape):
        dt = BF16 if name in BF_INPUTS else F32
        d[name] = nc.dram_tensor(name, list(shape), dt, kind="ExternalInput").ap()

    din("ximg", (128, 2, 66 * 66))
    din("xqT", (128, 2, NT))
    din("srdgi", (128, 2, 9, 128))
    din("updgi", (128, 2, 4, 9, 128))
    din("qwt", (C, C)); din("qb", (128, 2))
    din("kwt", (C, C)); din("kb", (128, 2))
    din("vwt", (C, C)); din("vb", (1, C)); din("vbp", (128, 2))
    din("srb", (128, 2))
    din("upb", (128, 2, 4))
    din("pwt", (C, C)); din("pwtu", (C, C)); din("pb", (1, C))
    din("wcsn", (1, C))
    din("sel", (2, 128))
    out = nc.dram_tensor("out", [NT, C], F32, kind="ExternalOutput").ap()

    with tile.TileContext(nc) as tc:
        with nc.allow_low_precision(reason="bf16 matmul operand pipeline"):
            body(nc, tc, d, out, half)
    nc.compile()
    return nc


def body(nc, tc, d, out, half):
    from contextlib import ExitStack
    ctx = ExitStack()
    with ctx:
        consts = ctx.enter_context(tc.tile_pool(name="consts", bufs=1))
        bigA = ctx.enter_context(tc.tile_pool(name="bigA", bufs=2))   # ximg / u
        bigB = ctx.enter_context(tc.tile_pool(name="bigB", bufs=1))   # xqT -> o_out
        persist = ctx.enter_context(tc.tile_pool(name="persist", bufs=1))
        diagp = ctx.enter_context(tc.tile_pool(name="diagp", bufs=1))
        expp = ctx.enter_context(tc.tile_pool(name="expp", bufs=5))
        smalls = ctx.enter_context(tc.tile_pool(name="smalls", bufs=2))
        sq = ctx.enter_context(tc.tile_pool(name="sq", bufs=2))
        outp = ctx.enter_context(tc.tile_pool(name="outp", bufs=2))
        psBig = ctx.enter_context(
            tc.tile_pool(name="psBig", bufs=2, space="PSUM"))   # [128,1024] f32
        psMid = ctx.enter_context(
            tc.tile_pool(name="psMid", bufs=2, space="PSUM"))   # [128,512] f32
        psPo = ctx.enter_context(
            tc.tile_pool(name="psPo", bufs=2, space="PSUM"))    # [65,512] f32

        # ---------- consts / inputs ----------
        def cload(name, shape, ap=None):
            dt = BF16 if name in BF_INPUTS else F32
            t = consts.tile(shape, dt, tag=name, name=f"c_{name}")
            nc.sync.dma_start(out=t[:], in_=ap if ap is not None else d[name])
            return t

        ones_col = consts.tile([128, 1], BF16, tag="ones_col")
        nc.gpsimd.memset(ones_col[:], 1.0)
        ones_row = consts.tile([1, 128], BF16, tag="ones_row")
        nc.gpsimd.memset(ones_row[:], 1.0)
        zbias = consts.tile([128, 1], F32, tag="zbias")
        nc.gpsimd.memset(zbias[:], 0.0)
        ebias = consts.tile([1, 1], F32, tag="ebias")
        nc.gpsimd.memset(ebias[:], EPS)
        zbias1 = consts.tile([1, 1], F32, tag="zbias1")
        nc.gpsimd.memset(zbias1[:], 0.0)
        ebias128 = consts.tile([128, 1], F32, tag="ebias128")
        nc.gpsimd.memset(ebias128[:], EPS)

        # ---------- persistent tiles ----------
        ximg = [bigA.tile([128, 66 * 66], BF16, tag="bigA", name=f"ximg{i}")
                for i in range(2)]
        xqT = bigB.tile([128, 2 * NT], BF16, tag="bigB")
        qT = persist.tile([128, 2, NT], BF16, tag="qT")
        kT = persist.tile([128, 2, M], BF16, tag="kT")
        xkvT = persist.tile([128, 2, M], BF16, tag="xkvT")
        v_aug = persist.tile([128, 8, 272], F8, tag="v_aug")
        vimg = persist.tile([128, 2, 34 * 34], BF16, tag="vimg")
        oT = persist.tile([128, 2, NT], BF16, tag="oT")
        rs_inv = persist.tile([1, 4 * NT], BF16, tag="rs_inv")
        ruT = persist.tile([128, 16], F32, tag="ruT")
        rm_rows = persist.tile([1, NT], BF16, tag="rm_rows")
        ident = consts.tile([128, 128], BF16, tag="ident")
        make_identity(nc, ident[:])

        # input DMAs, ordered so sr-conv (srdg+ximg) can start first, then the
        # LN/k chain (srb, kwt), then the q chain, then late-needed weights.
        srdg = diagp.tile([128, 2, 9, 128], BF16, tag="srdg")
        nc.sync.dma_start(out=srdg[:], in_=d["srdgi"])
        for cb in range(2):
            nc.sync.dma_start(out=ximg[cb][:], in_=d["ximg"][:, cb, :])
        srb = cload("srb", [128, 2])
        kwt = cload("kwt", [128, 2, C], d["kwt"].rearrange("(a p) c -> p a c", a=2))
        vwt = cload("vwt", [128, 2, C], d["vwt"].rearrange("(a p) c -> p a c", a=2))
        qb = cload("qb", [128, 2]); kb = cload("kb", [128, 2])
        vb = cload("vb", [1, C]); vbp = cload("vbp", [128, 2])
        nc.sync.dma_start(out=xqT[:].rearrange("p (a c) -> p a c", a=2),
                          in_=d["xqT"])
        qwt = cload("qwt", [128, 2, C], d["qwt"].rearrange("(a p) c -> p a c", a=2))
        sel0 = cload("sel", [1, 128], d["sel"][0:1, :])
        sel1 = consts.tile([1, 128], BF16, tag="sel1", name="c_sel1")
        nc.sync.dma_start(out=sel1[:], in_=d["sel"][1:2, :])
        pwt = cload("pwt", [128, 2, C], d["pwt"].rearrange("(a p) c -> p a c", a=2))
        pwtu = cload("pwtu", [128, 2, C], d["pwtu"].rearrange("(a p) c -> p a c", a=2))
        pb = cload("pb", [1, C])
        wcsn = cload("wcsn", [1, C])
        upb = cload("upb", [128, 2, 4])
        updg = diagp.tile([128, 2, 4, 9, 128], BF16, tag="updg")
        nc.sync.dma_start(out=updg[:], in_=d["updgi"])

        # vimg border zeroing (interior fully overwritten by vT-proj)
        for vo in range(2):
            vw = vimg[:, vo, :].rearrange("p (q c) -> p q c", q=34)
            nc.gpsimd.memset(vw[:, 0, :], 0.0)
            nc.gpsimd.memset(vw[:, 33, :], 0.0)
            nc.gpsimd.memset(vw[:, :, 0], 0.0)
            nc.gpsimd.memset(vw[:, :, 33], 0.0)
        ones_cols_view = bass.AP(
            tensor=v_aug.tensor, offset=v_aug.offset + 64,
            ap=[v_aug.ap[0], [272, 8], [65, 4], [1, 1]])
        nc.gpsimd.memset(ones_cols_view, 1.0)

        # ---------- sr depthwise conv (k=3, stride 2, pad 1) ----------
        x2s = {}
        for cb in range(2):
            pa = psBig.tile([128, M], F32, tag="big", name=f"pa{cb}")
            xv = ximg[cb][:].rearrange(
                "p (q a w b) -> p q a w b", q=33, a=2, b=2)
            for t in range(9):
                di, dj = t // 3, t % 3
                tap = xv[:, di // 2:di // 2 + 32, di % 2,
                         dj // 2:dj // 2 + 32, dj % 2]
                for ch in range(2):
                    nc.tensor.matmul(
                        pa[:, ch * 512:(ch + 1) * 512],
                        srdg[:, cb, t, :],
                        tap[:, ch * 16:(ch + 1) * 16, :],
                        start=(t == 0), stop=(t == 8))
            x2s[cb] = sq.tile([128, M], BF16, tag="squ", name=f"x2{cb}")
            for ch in range(2):
                sl = slice(ch * 512, (ch + 1) * 512)
                nc.scalar.activation(xkvT[:, cb, sl], pa[:, sl], AF.Identity,
                                     bias=srb[:, cb:cb + 1])
                nc.scalar.activation(x2s[cb][:, sl], pa[:, sl], AF.Square,
                                     bias=srb[:, cb:cb + 1])

        # ---------- LN over C for x_kv (g/be folded into kv weights) ----------
        for ch in range(2):
            sl = slice(ch * 512, (ch + 1) * 512)
            sx = psBig.tile([1, 512], F32, tag="big", name=f"sx{ch}")
            sx2 = psBig.tile([1, 512], F32, tag="big", name=f"sx2{ch}")
            for cb in range(2):
                nc.tensor.matmul(sx[:], ones_col[:], xkvT[:, cb, sl],
                                 start=(cb == 0), stop=(cb == 1))
                nc.tensor.matmul(sx2[:], ones_col[:], x2s[cb][:, sl],
                                 start=(cb == 0), stop=(cb == 1))
            mean = smalls.tile([1, 512], BF16, tag="mean", name=f"xmean{ch}", bufs=5)
            rstd = smalls.tile([1, 512], BF16, tag="rstd", name=f"xrstd{ch}", bufs=5)
            sA = smalls.tile([1, 512], F32, tag="sA", name=f"xsA{ch}")
            sB = smalls.tile([1, 512], F32, tag="sB", name=f"xsB{ch}")
            nc.vector.tensor_scalar_mul(mean[:], sx[:], 1.0 / C)
            nc.vector.tensor_mul(sB[:], mean[:], mean[:])
            nc.vector.scalar_tensor_tensor(sA[:], sx2[:], 1.0 / C, sB[:],
                                           op0=ALU.mult, op1=ALU.subtract)
            nc.scalar.activation(rstd[:], sA[:], AF.Abs_reciprocal_sqrt,
                                 bias=ebias[:])
            mb = psBig.tile([128, 512], F32, tag="big", name=f"mb{ch}")
            rb = psBig.tile([128, 512], F32, tag="big", name=f"rb{ch}")
            nc.tensor.matmul(mb[:], ones_row[:], mean[:])
            nc.tensor.matmul(rb[:], ones_row[:], rstd[:])
            for cb in range(2):
                nc.vector.tensor_sub(xkvT[:, cb, sl], xkvT[:, cb, sl], mb[:])
                nc.vector.tensor_mul(xkvT[:, cb, sl], xkvT[:, cb, sl], rb[:])

        # ---------- q projection ----------
        for dq in range(2):
            for cq in range(2):
                pq = psBig.tile([128, 1024], F32, tag="big", name=f"pq{dq}{cq}")
                for cb in range(2):
                    for ch in range(2):
                        so = slice(ch * 512, (ch + 1) * 512)
                        nc.tensor.matmul(
                            pq[:, so], qwt[:, cb, dq * 128:(dq + 1) * 128],
                            xqT[:, cb * NT + cq * 1024 + ch * 512:
                                cb * NT + cq * 1024 + (ch + 1) * 512],
                            start=(cb == 0), stop=(cb == 1))
                nc.scalar.activation(qT[:, dq, cq * 1024:(cq + 1) * 1024], pq[:],
                                     AF.Identity, bias=qb[:, dq:dq + 1])

        # ---------- k / v projections ----------
        for ko in range(2):
            pk = psBig.tile([128, M], F32, tag="big", name=f"pk{ko}")
            for cb in range(2):
                for ch in range(2):
                    sl = slice(ch * 512, (ch + 1) * 512)
                    nc.tensor.matmul(
                        pk[:, sl], kwt[:, cb, ko * 128:(ko + 1) * 128],
                        xkvT[:, cb, sl], start=(cb == 0), stop=(cb == 1))
            nc.scalar.activation(kT[:, ko, :], pk[:], AF.Identity,
                                 bias=kb[:, ko:ko + 1])
        for mt in range(8):
            pv = psMid.tile([128, 256], F32, tag="mid", name=f"pv{mt}")
            for cb in range(2):
                nc.tensor.matmul(pv[:], xkvT[:, cb, mt * 128:(mt + 1) * 128],
                                 vwt[:, cb, :], start=(cb == 0), stop=False)
            nc.tensor.matmul(pv[:], ones_row[:], vb[:],
                             start=False, stop=True)
            va_dst = bass.AP(
                tensor=v_aug.tensor, offset=v_aug.offset + mt * 272,
                ap=[v_aug.ap[0], [65, 4], [1, 64]])
            nc.vector.tensor_copy(va_dst, pv[:].rearrange("p (a b) -> p a b", a=4))

        # ---------- vT projection -> vimg (channels-major v) ----------
        for vo in range(2):
            pvt = psBig.tile([128, M], F32, tag="big", name=f"pvt{vo}")
            for cb in range(2):
                for ch in range(2):
                    sl = slice(ch * 512, (ch + 1) * 512)
                    nc.tensor.matmul(
                        pvt[:, sl], vwt[:, cb, vo * 128:(vo + 1) * 128],
                        xkvT[:, cb, sl], start=(cb == 0), stop=(cb == 1))
            dst = vimg[:, vo, :].rearrange("p (q c) -> p q c", q=34)[
                :, 1:33, 1:33]
            nc.scalar.activation(
                dst, pvt[:].rearrange("p (a b) -> p a b", a=32),
                AF.Identity, bias=vbp[:, vo:vo + 1])

        # ---------- attention + interleaved up-branch (own half only) ----------
        ustats = {}
        vv = vimg[:].rearrange("p a (q w) -> p a q w", q=34)
        u = bigA.tile([128, 2, NT], BF16, tag="bigA", name="u")

        def up_conv_block(cb, pl):
            dg = updg[:, cb, pl]
            pu = psMid.tile([128, 512], F32, tag="mid", name=f"pu{cb}{pl}")
            for t in range(9):
                di, dj = t // 3, t % 3
                tap = vv[:, cb, di + 16 * half:di + 16 * half + 16, dj:dj + 32]
                nc.tensor.matmul(pu[:], dg[:, t, :], tap,
                                 start=(t == 0), stop=(t == 8))
            r1, r2 = pl // 2, pl % 2
            dst = u[:, cb, :].rearrange(
                "p (a x b y) -> p a x b y", a=16, x=2, y=2)[:, :, r1, :, r2]
            nc.vector.tensor_scalar_add(dst, pu[:].rearrange(
                "p (a b) -> p a b", a=16), upb[:, cb, pl:pl + 1])

        def u_ln_stats():
            # squares on Pool (idle in the tail); channel-sums directly in
            # token-partition layout via data-as-lhsT matmuls (out free = 1).
            u2s = []
            for cb in range(2):
                u2 = sq.tile([128, NT], BF16, tag="squ", name=f"u2{cb}")
                nc.vector.tensor_mul(u2[:], u[:, cb, :], u[:, cb, :])
                u2s.append(u2)
            # start=True resets the WHOLE psum bank, so pre-zero once and
            # accumulate everything with start=False
            su = psMid.tile([128, 2, 16], F32, tag="mid", name="suT")
            nc.vector.memset(su[:], 0.0)
            for ntl in range(16):
                tb = slice(ntl * 128, (ntl + 1) * 128)
                for cb in range(2):
                    nc.tensor.matmul(su[:, 0, ntl:ntl + 1], u[:, cb, tb],
                                     ones_col[:], start=False,
                                     stop=(cb == 1), skip_group_check=True)
                for cb in range(2):
                    nc.tensor.matmul(su[:, 1, ntl:ntl + 1], u2s[cb][:, tb],
                                     ones_col[:], start=False,
                                     stop=(cb == 1), skip_group_check=True)
            meanT = smalls.tile([128, 16], F32, tag="mean", name="umeanT", bufs=5)
            m2T = smalls.tile([128, 16], F32, tag="sB", name="um2T")
            varT = smalls.tile([128, 16], F32, tag="sA", name="uvarT")
            rmT = smalls.tile([128, 16], BF16, tag="rstd", name="urmT", bufs=5)
            nc.vector.tensor_scalar_mul(meanT[:], su[:, 0, :], 1.0 / C)
            nc.vector.tensor_mul(m2T[:], meanT[:], meanT[:])
            nc.vector.scalar_tensor_tensor(varT[:], su[:, 1, :], 1.0 / C,
                                           m2T[:], op0=ALU.mult,
                                           op1=ALU.subtract)
            nc.scalar.activation(ruT[:], varT[:], AF.Abs_reciprocal_sqrt,
                                 bias=ebias128[:])
            # merge applies ru to the whole psum, so the rank-1 correction
            # carries only the mean (ru*(u@W + m*wcsn) = r*(u@W) - r*m*colsum)
            nc.vector.tensor_copy(rmT[:], meanT[:])
            # PE-transpose rmT to [16,128], then per-row evac to a single
            # partition so rank-1 lhsT slices have base partition 0
            prm = psMid.tile([16, 128], BF16, tag="mid", name="prm")
            nc.tensor.transpose(prm[:], rmT[:], ident[:])
            rm16 = smalls.tile([16, 128], BF16, tag="rm16", name="rm16")
            nc.vector.tensor_copy(rm16[:], prm[:])
            nc.sync.dma_start(
                out=rm_rows[0:1, :].rearrange("a (p c) -> a p c", p=16),
                in_=rm16[:])

        def o_norm(cb):
            for cq in range(2):
                rbo = psBig.tile([128, 1024], F32, tag="big", name=f"rbo{cb}{cq}")
                for ch in range(2):
                    so = slice(ch * 512, (ch + 1) * 512)
                    h0, h1 = 2 * cb, 2 * cb + 1
                    base = cq * 1024 + ch * 512
                    nc.tensor.matmul(rbo[:, so], sel0[:],
                                     rs_inv[:, h0 * NT + base:h0 * NT + base + 512],
                                     start=True, stop=False)
                    nc.tensor.matmul(rbo[:, so], sel1[:],
                                     rs_inv[:, h1 * NT + base:h1 * NT + base + 512],
                                     start=False, stop=True)
                nc.vector.tensor_mul(oT[:, cb, cq * 1024:(cq + 1) * 1024],
                                     oT[:, cb, cq * 1024:(cq + 1) * 1024], rbo[:])

        def u_proj_merge(ngs):
            # fused: psum pp[:,0,:] = u@pwtu + rm⊗wcsn, pp[:,1,:] = oTn@pwt+pb
            # merge: ot = pp_u * ru[token] + pp_o   (per-token scalar on DVE)
            for ng in ngs:
                ot = outp.tile([128, 4, 256], F32, tag="outt", name=f"ot{ng}")
                for j in range(4):
                    ntl = ng * 4 + j
                    tb = slice(ntl * 128, (ntl + 1) * 128)
                    ppu = psMid.tile([128, 256], F32, tag="mid",
                                     name=f"ppu{ng}{j}")
                    ppo = psMid.tile([128, 256], F32, tag="mid",
                                     name=f"ppo{ng}{j}")
                    for cb in range(2):
                        nc.tensor.matmul(ppu[:], u[:, cb, tb],
                                         pwtu[:, cb, :],
                                         start=(cb == 0), stop=False)
                    nc.tensor.matmul(ppu[:],
                                     rm_rows[0:1, ntl * 128:(ntl + 1) * 128],
                                     wcsn[:], start=False, stop=True)
                    for cb in range(2):
                        nc.tensor.matmul(ppo[:], oT[:, cb, tb],
                                         pwt[:, cb, :],
                                         start=(cb == 0), stop=False)
                    nc.tensor.matmul(ppo[:], ones_row[:], pb[:],
                                     start=False, stop=True)
                    # ACT evac applies the per-token LN scale; DVE adds o-psum
                    # (walrus rejects two PSUM tensor operands in one DVE op)
                    usc = sq.tile([128, 256], BF16, tag="ppsb",
                                  name=f"usc{ng}{j}")
                    nc.scalar.activation(usc[:], ppu[:], AF.Identity,
                                         bias=zbias[:],
                                         scale=ruT[:, ntl:ntl + 1])
                    nc.vector.tensor_add(ot[:, j, :], usc[:], ppo[:])
                nt0 = ng * 4
                nc.sync.dma_start(
                    out=out[nt0 * 128:(nt0 + 4) * 128, :].rearrange(
                        "(a p) c -> p a c", a=4),
                    in_=ot[:])

        def attention_head(h):
            cb, hr = h // 2, (h % 2) * 64
            est = []

            def qk_exp(pt):
                e = expp.tile([128, 2, NT], F8, tag="expst", name=f"e{h}_{pt}")
                est.append(e)
                for par in range(2):
                    for cq in range(2):
                        st = psBig.tile([128, 1024], F32, tag="big",
                                        name=f"st{h}_{pt}_{par}_{cq}")
                        mt = pt * 2 + par
                        for ch in range(2):
                            nc.tensor.matmul(
                                st[:, ch * 512:(ch + 1) * 512],
                                kT[hr:hr + 64, cb, mt * 128:(mt + 1) * 128],
                                qT[hr:hr + 64, cb,
                                   cq * 1024 + ch * 512:cq * 1024 + (ch + 1) * 512])
                        nc.scalar.activation(
                            e[:, par, cq * 1024:(cq + 1) * 1024], st[:],
                            AF.Exp, bias=zbias[:], scale=SCALE)

            for pt in range(4):
                qk_exp(pt)
            for nq in range(4):
                po = psPo.tile([65, 512], F32, tag="po", name=f"po{h}{nq}")
                sl = slice(nq * 512, (nq + 1) * 512)
                for pt in range(4):
                    nc.tensor.matmul(po[:],
                                     v_aug[:, 2 * pt:2 * pt + 2,
                                           65 * h:65 * h + 65],
                                     est[pt][:, :, sl],
                                     start=(pt == 0), stop=(pt == 3),
                                     perf_mode=mybir.MatmulPerfMode.DoubleRow)
                nc.vector.reciprocal(
                    rs_inv[:, h * NT + nq * 512:h * NT + (nq + 1) * 512],
                    po[64:65, :])
                nc.vector.tensor_copy(
                    oT[hr:hr + 64, cb, nq * 512:(nq + 1) * 512], po[0:64, :])

        for h in range(HEADS):
            attention_head(h)
            up_conv_block(0, h)
            up_conv_block(1, h)
            if h == 1:
                o_norm(0)
        o_norm(1)
        u_ln_stats()
        u_proj_merge([0, 1, 2, 3])


_NC_CACHE = {}


def get_nc(half=0):
    if half not in _NC_CACHE:
        _NC_CACHE[half] = build_nc(half)
    return _NC_CACHE[half]


def host_prep(q_w, q_b, kv_w, kv_b, sr_w, sr_b, sr_g, sr_be,
              up_w, up_b, upn_g, upn_be, proj_w, proj_b):
    f32 = np.float32
    w = {}
    w["qwt"] = q_w.T
    w["qb"] = q_b.reshape(2, 128).T
    kwt = kv_w[:C].T * sr_g[:, None]
    vwt = kv_w[C:].T * sr_g[:, None]
    w["kwt"] = kwt
    w["vwt"] = vwt
    w["kb"] = (kv_b[:C] + sr_be @ kwt).reshape(2, 128).T
    vbe = kv_b[C:] + sr_be @ vwt
    w["vb"] = vbe.reshape(1, C)
    w["vbp"] = vbe.reshape(2, 128).T
    w["srb"] = sr_b.reshape(2, 128).T
    w["upb"] = up_b.reshape(C, 4).reshape(2, 128, 4).transpose(1, 0, 2)
    pwt = proj_w.T
    w["pwt"] = pwt
    w["pwtu"] = pwt * upn_g[:, None]
    w["pb"] = (proj_b + upn_be @ pwt).reshape(1, C)
    w["wcsn"] = (-w["pwtu"].sum(axis=0)).reshape(1, C)
    sel = np.zeros((2, 128), f32)
    sel[0, :64] = 1.0
    sel[1, 64:] = 1.0
    w["sel"] = sel

    # diagonal-expanded depthwise conv weights
    srw = sr_w.reshape(C, 9)                      # [c, t]
    srdgi = np.zeros((128, 2, 9, 128), f32)
    srdgi[np.arange(128), :, :, np.arange(128)] = \
        srw.reshape(2, 128, 9).transpose(1, 0, 2)
    w["srdgi"] = srdgi
    upw = up_w.reshape(C, 4, 9)                   # [c, pl, t]
    updgi = np.zeros((128, 2, 4, 9, 128), f32)
    updgi[np.arange(128), :, :, :, np.arange(128)] = \
        upw.reshape(2, 128, 4, 9).transpose(1, 0, 2, 3)
    w["updgi"] = updgi

    res = {}
    for k, v in w.items():
        dt = ml_dtypes.bfloat16 if k in BF_INPUTS else f32
        res[k] = np.ascontiguousarray(np.asarray(v, f32).astype(dt))
    return res


def make_in_maps(x, w):
    bf = ml_dtypes.bfloat16
    ximgs = []
    for b in range(B):
        xi = x[b].T.reshape(2, 128, H, W)          # [cb, p, 64, 64]
        pad = np.zeros((2, 128, H + 2, W + 2), np.float32)
        pad[:, :, 1:H + 1, 1:W + 1] = xi
        ximgs.append(np.ascontiguousarray(
            pad.transpose(1, 0, 2, 3).reshape(128, 2, 66 * 66).astype(bf)))
    in_maps = []
    for core in range(8):
        b, half = core // 2, core % 2
        m = dict(w)
        m["ximg"] = ximgs[b]
        xh = x[b, half * NT:(half + 1) * NT]       # [NT, C]
        m["xqT"] = np.ascontiguousarray(
            xh.T.reshape(2, 128, NT).transpose(1, 0, 2).astype(bf))
        in_maps.append(m)
    return in_maps


def kernel(x, q_w, q_b, kv_w, kv_b, sr_w, sr_b, sr_g, sr_be,
           up_w, up_b, upn_g, upn_be, proj_w, proj_b, H, W):
    assert int(H) == 64 and int(W) == 64
    f32 = np.float32
    x = np.asarray(x, f32)
    w = host_prep(np.asarray(q_w, f32), np.asarray(q_b, f32),
                  np.asarray(kv_w, f32), np.asarray(kv_b, f32),
                  np.asarray(sr_w, f32), np.asarray(sr_b, f32),
                  np.asarray(sr_g, f32), np.asarray(sr_be, f32),
                  np.asarray(up_w, f32), np.asarray(up_b, f32),
                  np.asarray(upn_g, f32), np.asarray(upn_be, f32),
                  np.asarray(proj_w, f32), np.asarray(proj_b, f32))
    in_maps = make_in_maps(x, w)
    out = np.empty((B, N, C), f32)
    for half in range(2):
        nc = get_nc(half)
        maps = [in_maps[2 * b + half] for b in range(B)]
        res = None
        for attempt in range(3):
            try:
                res = run_bass_kernel_spmd(nc, maps, core_ids=list(range(4))).results
                break
            except Exception:
                if attempt == 2:
                    raise
        assert res is not None
        for b in range(B):
            out[b, half * NT:(half + 1) * NT] = res[b]["out"]
    return out


# revision 47
# speedup vs baseline: 1.0060x; 1.0060x over previous
"""Trainium2 Bass kernel for nn_Attention_65180423685043 (sparse_attention).

Sharding: 8 cores = 4 batches x 2 sequence-halves. Each core computes the
spatial-reduction/kv branch for its batch (duplicated within the pair) and
attention + the up branch only for its 2048-token half. No collectives.

Host prep ships layout-transformed inputs (no arithmetic on x beyond dtype
cast): channel-major padded image ximg, transposed xqT, and diagonal-expanded
depthwise conv weights. Two compiled variants exist because the up-conv tap
row offset (16*half) is a compile-time AP constant.
"""
import numpy as np
import ml_dtypes

import concourse.bass as bass
import concourse.mybir as mybir
import concourse.tile as tile
from concourse import bacc
from concourse.bass_utils import run_bass_kernel_spmd
from concourse.masks import make_identity

F32 = mybir.dt.float32
BF16 = mybir.dt.bfloat16
F8 = mybir.dt.float8e4
AF = mybir.ActivationFunctionType
ALU = mybir.AluOpType

B, N, C, HEADS, SR = 4, 4096, 256, 4, 2
HD = C // HEADS
H = W = 64
Hs = Ws = 32
M = Hs * Ws          # kv tokens
NT = N // 2          # tokens per core
SCALE = HD ** -0.5
EPS = 1e-6

BF_INPUTS = {"ximg", "xqT", "srdgi", "updgi", "qwtf", "kwtf", "vwt",
             "pwt", "pwtu", "vb", "pb", "sel", "wcsn"}


def build_nc(half):
    nc = bacc.Bacc("TRN2", target_bir_lowering=False, debug=False, num_devices=8)
    d = {}

    def din(name, shape):
        dt = BF16 if name in BF_INPUTS else F32
        d[name] = nc.dram_tensor(name, list(shape), dt, kind="ExternalInput").ap()

    din("ximg", (128, 2, 66 * 66))
    din("xqT", (128, 2, NT))
    din("srdgi", (128, 2, 9, 128))
    din("updgi", (128, 2, 4, 9, 128))
    din("qwtf", (128, 2, 2, 128)); din("qbf", (128, 2))
    din("kwtf", (128, 2, 2, 128)); din("kbf", (128, 2))
    din("vwt", (C, C)); din("vb", (1, C)); din("vbp", (128, 2))
    din("srb", (128, 2))
    din("upb", (128, 2, 4))
    din("pwt", (C, C)); din("pwtu", (C, C)); din("pb", (1, C))
    din("wcsn", (1, C))
    din("sel", (2, 128))
    out = nc.dram_tensor("out", [NT, C], F32, kind="ExternalOutput").ap()

    with tile.TileContext(nc) as tc:
        with nc.allow_low_precision(reason="bf16 matmul operand pipeline"):
            body(nc, tc, d, out, half)
    nc.compile()
    return nc


def body(nc, tc, d, out, half):
    from contextlib import ExitStack
    ctx = ExitStack()
    with ctx:
        consts = ctx.enter_context(tc.tile_pool(name="consts", bufs=1))
        bigA = ctx.enter_context(tc.tile_pool(name="bigA", bufs=2))   # ximg / u
        bigB = ctx.enter_context(tc.tile_pool(name="bigB", bufs=1))   # xqT -> o_out
        persist = ctx.enter_context(tc.tile_pool(name="persist", bufs=1))
        diagp = ctx.enter_context(tc.tile_pool(name="diagp", bufs=1))
        expp = ctx.enter_context(tc.tile_pool(name="expp", bufs=5))
        smalls = ctx.enter_context(tc.tile_pool(name="smalls", bufs=2))
        sq = ctx.enter_context(tc.tile_pool(name="sq", bufs=2))
        outp = ctx.enter_context(tc.tile_pool(name="outp", bufs=2))
        psBig = ctx.enter_context(
            tc.tile_pool(name="psBig", bufs=2, space="PSUM"))   # [128,1024] f32
        psMid = ctx.enter_context(
            tc.tile_pool(name="psMid", bufs=2, space="PSUM"))   # [128,512] f32
        psPo = ctx.enter_context(
            tc.tile_pool(name="psPo", bufs=2, space="PSUM"))    # [65,512] f32

        # ---------- consts / inputs ----------
        def cload(name, shape, ap=None):
            dt = BF16 if name in BF_INPUTS else F32
            t = consts.tile(shape, dt, tag=name, name=f"c_{name}")
            nc.sync.dma_start(out=t[:], in_=ap if ap is not None else d[name])
            return t

        ones_col = consts.tile([128, 1], BF16, tag="ones_col")
        nc.gpsimd.memset(ones_col[:], 1.0)
        ones_row = consts.tile([1, 128], BF16, tag="ones_row")
        nc.gpsimd.memset(ones_row[:], 1.0)
        zbias = consts.tile([128, 1], F32, tag="zbias")
        nc.gpsimd.memset(zbias[:], 0.0)
        ebias = consts.tile([1, 1], F32, tag="ebias")
        nc.gpsimd.memset(ebias[:], EPS)
        zbias1 = consts.tile([1, 1], F32, tag="zbias1")
        nc.gpsimd.memset(zbias1[:], 0.0)
        ebias128 = consts.tile([128, 1], F32, tag="ebias128")
        nc.gpsimd.memset(ebias128[:], EPS)

        # ---------- persistent tiles ----------
        ximg = [bigA.tile([128, 66 * 66], BF16, tag="bigA", name=f"ximg{i}")
                for i in range(2)]
        xqT = bigB.tile([128, 2 * NT], BF16, tag="bigB")
        qF = persist.tile([128, 2, NT], F8, tag="qF")
        kF = persist.tile([128, 2, M], F8, tag="kF")
        xkvT = persist.tile([128, 2, M], BF16, tag="xkvT")
        v_aug = persist.tile([128, 8, 272], F8, tag="v_aug")
        vimg = persist.tile([128, 2, 34 * 34], BF16, tag="vimg")
        oT = persist.tile([128, 2, NT], BF16, tag="oT")
        rs_inv = persist.tile([1, 4 * NT], BF16, tag="rs_inv")
        ruT = persist.tile([128, 16], F32, tag="ruT")
        rm_rows = persist.tile([1, NT], BF16, tag="rm_rows")
        ident = consts.tile([128, 128], BF16, tag="ident")
        make_identity(nc, ident[:])

        # input DMAs, ordered so sr-conv (srdg+ximg) can start first, then the
        # LN/k chain (srb, kwt), then the q chain, then late-needed weights.
        srdg = diagp.tile([128, 2, 9, 128], BF16, tag="srdg")
        nc.sync.dma_start(out=srdg[:], in_=d["srdgi"])
        for cb in range(2):
            nc.sync.dma_start(out=ximg[cb][:], in_=d["ximg"][:, cb, :])
        srb = cload("srb", [128, 2])
        kwtf = cload("kwtf", [128, 2, 2, 128])
        vwt = cload("vwt", [128, 2, C], d["vwt"].rearrange("(a p) c -> p a c", a=2))
        qbf = cload("qbf", [128, 2]); kbf = cload("kbf", [128, 2])
        vb = cload("vb", [1, C]); vbp = cload("vbp", [128, 2])
        nc.sync.dma_start(out=xqT[:].rearrange("p (a c) -> p a c", a=2),
                          in_=d["xqT"])
        qwtf = cload("qwtf", [128, 2, 2, 128])
        sel0 = cload("sel", [1, 128], d["sel"][0:1, :])
        sel1 = consts.tile([1, 128], BF16, tag="sel1", name="c_sel1")
        nc.sync.dma_start(out=sel1[:], in_=d["sel"][1:2, :])
        pwt = cload("pwt", [128, 2, C], d["pwt"].rearrange("(a p) c -> p a c", a=2))
        pwtu = cload("pwtu", [128, 2, C], d["pwtu"].rearrange("(a p) c -> p a c", a=2))
        pb = cload("pb", [1, C])
        wcsn = cload("wcsn", [1, C])
        upb = cload("upb", [128, 2, 4])
        updg = diagp.tile([128, 2, 4, 9, 128], BF16, tag="updg")
        nc.sync.dma_start(out=updg[:], in_=d["updgi"])

        # vimg border zeroing (interior fully overwritten by vT-proj)
        for vo in range(2):
            vw = vimg[:, vo, :].rearrange("p (q c) -> p q c", q=34)
            nc.gpsimd.memset(vw[:, 0, :], 0.0)
            nc.gpsimd.memset(vw[:, 33, :], 0.0)
            nc.gpsimd.memset(vw[:, :, 0], 0.0)
            nc.gpsimd.memset(vw[:, :, 33], 0.0)
        ones_cols_view = bass.AP(
            tensor=v_aug.tensor, offset=v_aug.offset + 64,
            ap=[v_aug.ap[0], [272, 8], [65, 4], [1, 1]])
        nc.gpsimd.memset(ones_cols_view, 1.0)

        # ---------- sr depthwise conv (k=3, stride 2, pad 1) ----------
        x2s = {}
        for cb in range(2):
            pa = psBig.tile([128, M], F32, tag="big", name=f"pa{cb}")
            xv = ximg[cb][:].rearrange(
                "p (q a w b) -> p q a w b", q=33, a=2, b=2)
            for t in range(9):
                di, dj = t // 3, t % 3
                tap = xv[:, di // 2:di // 2 + 32, di % 2,
                         dj // 2:dj // 2 + 32, dj % 2]
                for ch in range(2):
                    nc.tensor.matmul(
                        pa[:, ch * 512:(ch + 1) * 512],
                        srdg[:, cb, t, :],
                        tap[:, ch * 16:(ch + 1) * 16, :],
                        start=(t == 0), stop=(t == 8))
            x2s[cb] = sq.tile([128, M], BF16, tag="squ", name=f"x2{cb}")
            for ch in range(2):
                sl = slice(ch * 512, (ch + 1) * 512)
                nc.scalar.activation(xkvT[:, cb, sl], pa[:, sl], AF.Identity,
                                     bias=srb[:, cb:cb + 1])
                nc.scalar.activation(x2s[cb][:, sl], pa[:, sl], AF.Square,
                                     bias=srb[:, cb:cb + 1])

        # ---------- LN over C for x_kv (g/be folded into kv weights) ----------
        for ch in range(2):
            sl = slice(ch * 512, (ch + 1) * 512)
            sx = psBig.tile([1, 512], F32, tag="big", name=f"sx{ch}")
            sx2 = psBig.tile([1, 512], F32, tag="big", name=f"sx2{ch}")
            for cb in range(2):
                nc.tensor.matmul(sx[:], ones_col[:], xkvT[:, cb, sl],
                                 start=(cb == 0), stop=(cb == 1))
                nc.tensor.matmul(sx2[:], ones_col[:], x2s[cb][:, sl],
                                 start=(cb == 0), stop=(cb == 1))
            mean = smalls.tile([1, 512], BF16, tag="mean", name=f"xmean{ch}", bufs=5)
            rstd = smalls.tile([1, 512], BF16, tag="rstd", name=f"xrstd{ch}", bufs=5)
            sA = smalls.tile([1, 512], F32, tag="sA", name=f"xsA{ch}")
            sB = smalls.tile([1, 512], F32, tag="sB", name=f"xsB{ch}")
            nc.vector.tensor_scalar_mul(mean[:], sx[:], 1.0 / C)
            nc.vector.tensor_mul(sB[:], mean[:], mean[:])
            nc.vector.scalar_tensor_tensor(sA[:], sx2[:], 1.0 / C, sB[:],
                                           op0=ALU.mult, op1=ALU.subtract)
            nc.scalar.activation(rstd[:], sA[:], AF.Abs_reciprocal_sqrt,
                                 bias=ebias[:])
            mb = psBig.tile([128, 512], F32, tag="big", name=f"mb{ch}")
            rb = psBig.tile([128, 512], F32, tag="big", name=f"rb{ch}")
            nc.tensor.matmul(mb[:], ones_row[:], mean[:])
            nc.tensor.matmul(rb[:], ones_row[:], rstd[:])
            for cb in range(2):
                nc.vector.tensor_sub(xkvT[:, cb, sl], xkvT[:, cb, sl], mb[:])
                nc.vector.tensor_mul(xkvT[:, cb, sl], xkvT[:, cb, sl], rb[:])

        # ---------- q projection (folded-parity planes, fp8 out) ----------
        for sp in range(2):
            for cq in range(2):
                pq = psBig.tile([128, 1024], F32, tag="big", name=f"pq{sp}{cq}")
                for cb in range(2):
                    for ch in range(2):
                        so = slice(ch * 512, (ch + 1) * 512)
                        nc.tensor.matmul(
                            pq[:, so], qwtf[:, cb, sp, :],
                            xqT[:, cb * NT + cq * 1024 + ch * 512:
                                cb * NT + cq * 1024 + (ch + 1) * 512],
                            start=(cb == 0), stop=(cb == 1))
                nc.scalar.activation(qF[:, sp, cq * 1024:(cq + 1) * 1024], pq[:],
                                     AF.Identity, bias=qbf[:, sp:sp + 1])

        # ---------- k / v projections ----------
        for sp in range(2):
            pk = psBig.tile([128, M], F32, tag="big", name=f"pk{sp}")
            for cb in range(2):
                for ch in range(2):
                    sl = slice(ch * 512, (ch + 1) * 512)
                    nc.tensor.matmul(
                        pk[:, sl], kwtf[:, cb, sp, :],
                        xkvT[:, cb, sl], start=(cb == 0), stop=(cb == 1))
            nc.scalar.activation(kF[:, sp, :], pk[:], AF.Identity,
                                 bias=kbf[:, sp:sp + 1])
        for mt in range(8):
            pv = psMid.tile([128, 256], F32, tag="mid", name=f"pv{mt}")
            for cb in range(2):
                nc.tensor.matmul(pv[:], xkvT[:, cb, mt * 128:(mt + 1) * 128],
                                 vwt[:, cb, :], start=(cb == 0), stop=False)
            nc.tensor.matmul(pv[:], ones_row[:], vb[:],
                             start=False, stop=True)
            va_dst = bass.AP(
                tensor=v_aug.tensor, offset=v_aug.offset + mt * 272,
                ap=[v_aug.ap[0], [65, 4], [1, 64]])
            nc.vector.tensor_copy(va_dst, pv[:].rearrange("p (a b) -> p a b", a=4))

        # ---------- vT projection -> vimg (channels-major v) ----------
        for vo in range(2):
            pvt = psBig.tile([128, M], F32, tag="big", name=f"pvt{vo}")
            for cb in range(2):
                for ch in range(2):
                    sl = slice(ch * 512, (ch + 1) * 512)
                    nc.tensor.matmul(
                        pvt[:, sl], vwt[:, cb, vo * 128:(vo + 1) * 128],
                        xkvT[:, cb, sl], start=(cb == 0), stop=(cb == 1))
            dst = vimg[:, vo, :].rearrange("p (q c) -> p q c", q=34)[
                :, 1:33, 1:33]
            nc.scalar.activation(
                dst, pvt[:].rearrange("p (a b) -> p a b", a=32),
                AF.Identity, bias=vbp[:, vo:vo + 1])

        # ---------- attention + interleaved up-branch (own half only) ----------
        ustats = {}
        vv = vimg[:].rearrange("p a (q w) -> p a q w", q=34)
        u = bigA.tile([128, 2, NT], BF16, tag="bigA", name="u")

        def up_conv_block(cb, pl):
            dg = updg[:, cb, pl]
            pu = psMid.tile([128, 512], F32, tag="mid", name=f"pu{cb}{pl}")
            for t in range(9):
                di, dj = t // 3, t % 3
                tap = vv[:, cb, di + 16 * half:di + 16 * half + 16, dj:dj + 32]
                nc.tensor.matmul(pu[:], dg[:, t, :], tap,
                                 start=(t == 0), stop=(t == 8))
            r1, r2 = pl // 2, pl % 2
            dst = u[:, cb, :].rearrange(
                "p (a x b y) -> p a x b y", a=16, x=2, y=2)[:, :, r1, :, r2]
            nc.vector.tensor_scalar_add(dst, pu[:].rearrange(
                "p (a b) -> p a b", a=16), upb[:, cb, pl:pl + 1])

        def u_ln_stats():
            # squares on Pool (idle in the tail); channel-sums directly in
            # token-partition layout via data-as-lhsT matmuls (out free = 1).
            u2s = []
            for cb in range(2):
                u2 = sq.tile([128, NT], BF16, tag="squ", name=f"u2{cb}")
                nc.vector.tensor_mul(u2[:], u[:, cb, :], u[:, cb, :])
                u2s.append(u2)
            # start=True resets the WHOLE psum bank, so pre-zero once and
            # accumulate everything with start=False
            su = psMid.tile([128, 2, 16], F32, tag="mid", name="suT")
            nc.vector.memset(su[:], 0.0)
            for ntl in range(16):
                tb = slice(ntl * 128, (ntl + 1) * 128)
                for cb in range(2):
                    nc.tensor.matmul(su[:, 0, ntl:ntl + 1], u[:, cb, tb],
                                     ones_col[:], start=False,
                                     stop=(cb == 1), skip_group_check=True)
                for cb in range(2):
                    nc.tensor.matmul(su[:, 1, ntl:ntl + 1], u2s[cb][:, tb],
                                     ones_col[:], start=False,
                                     stop=(cb == 1), skip_group_check=True)
            meanT = smalls.tile([128, 16], F32, tag="mean", name="umeanT", bufs=5)
            m2T = smalls.tile([128, 16], F32, tag="sB", name="um2T")
            varT = smalls.tile([128, 16], F32, tag="sA", name="uvarT")
            rmT = smalls.tile([128, 16], BF16, tag="rstd", name="urmT", bufs=5)
            nc.vector.tensor_scalar_mul(meanT[:], su[:, 0, :], 1.0 / C)
            nc.vector.tensor_mul(m2T[:], meanT[:], meanT[:])
            nc.vector.scalar_tensor_tensor(varT[:], su[:, 1, :], 1.0 / C,
                                           m2T[:], op0=ALU.mult,
                                           op1=ALU.subtract)
            nc.scalar.activation(ruT[:], varT[:], AF.Abs_reciprocal_sqrt,
                                 bias=ebias128[:])
            # merge applies ru to the whole psum, so the rank-1 correction
            # carries only the mean (ru*(u@W + m*wcsn) = r*(u@W) - r*m*colsum)
            nc.vector.tensor_copy(rmT[:], meanT[:])
            # PE-transpose rmT to [16,128], then per-row evac to a single
            # partition so rank-1 lhsT slices have base partition 0
            prm = psMid.tile([16, 128], BF16, tag="mid", name="prm")
            nc.tensor.transpose(prm[:], rmT[:], ident[:])
            rm16 = smalls.tile([16, 128], BF16, tag="rm16", name="rm16")
            nc.vector.tensor_copy(rm16[:], prm[:])
            nc.sync.dma_start(
                out=rm_rows[0:1, :].rearrange("a (p c) -> a p c", p=16),
                in_=rm16[:])

        def o_norm(cb):
            for cq in range(2):
                rbo = psBig.tile([128, 1024], F32, tag="big", name=f"rbo{cb}{cq}")
                for ch in range(2):
                    so = slice(ch * 512, (ch + 1) * 512)
                    h0, h1 = 2 * cb, 2 * cb + 1
                    base = cq * 1024 + ch * 512
                    nc.tensor.matmul(rbo[:, so], sel0[:],
                                     rs_inv[:, h0 * NT + base:h0 * NT + base + 512],
                                     start=True, stop=False)
                    nc.tensor.matmul(rbo[:, so], sel1[:],
                                     rs_inv[:, h1 * NT + base:h1 * NT + base + 512],
                                     start=False, stop=True)
                nc.vector.tensor_mul(oT[:, cb, cq * 1024:(cq + 1) * 1024],
                                     oT[:, cb, cq * 1024:(cq + 1) * 1024], rbo[:])

        def u_proj_merge(ngs):
            # fused: psum pp[:,0,:] = u@pwtu + rm⊗wcsn, pp[:,1,:] = oTn@pwt+pb
            # merge: ot = pp_u * ru[token] + pp_o   (per-token scalar on DVE)
            for ng in ngs:
                ot = outp.tile([128, 4, 256], F32, tag="outt", name=f"ot{ng}")
                for j in range(4):
                    ntl = ng * 4 + j
                    tb = slice(ntl * 128, (ntl + 1) * 128)
                    ppu = psMid.tile([128, 256], F32, tag="mid",
                                     name=f"ppu{ng}{j}")
                    ppo = psMid.tile([128, 256], F32, tag="mid",
                                     name=f"ppo{ng}{j}")
                    for cb in range(2):
                        nc.tensor.matmul(ppu[:], u[:, cb, tb],
                                         pwtu[:, cb, :],
                                         start=(cb == 0), stop=False)
                    nc.tensor.matmul(ppu[:],
                                     rm_rows[0:1, ntl * 128:(ntl + 1) * 128],
                                     wcsn[:], start=False, stop=True)
                    for cb in range(2):
                        nc.tensor.matmul(ppo[:], oT[:, cb, tb],
                                         pwt[:, cb, :],
                                         start=(cb == 0), stop=False)
                    nc.tensor.matmul(ppo[:], ones_row[:], pb[:],
                                     start=False, stop=True)
                    # ACT evac applies the per-token LN scale; DVE adds o-psum
                    # (walrus rejects two PSUM tensor operands in one DVE op)
                    usc = sq.tile([128, 256], BF16, tag="ppsb",
                                  name=f"usc{ng}{j}")
                    nc.scalar.activation(usc[:], ppu[:], AF.Identity,
                                         bias=zbias[:],
                                         scale=ruT[:, ntl:ntl + 1])
                    nc.vector.tensor_add(ot[:, j, :], usc[:], ppo[:])
                nt0 = ng * 4
                nc.sync.dma_start(
                    out=out[nt0 * 128:(nt0 + 4) * 128, :].rearrange(
                        "(a p) c -> p a c", a=4),
                    in_=ot[:])

        def attention_head(h):
            cb, hr = h // 2, (h % 2) * 64
            est = []

            def qk_exp(pt):
                e = expp.tile([128, 2, NT], F8, tag="expst", name=f"e{h}_{pt}")
                est.append(e)
                hp = slice(32 * h, 32 * h + 32)
                for par in range(2):
                    for cq in range(2):
                        st = psBig.tile([128, 1024], F32, tag="big",
                                        name=f"st{h}_{pt}_{par}_{cq}")
                        mt = pt * 2 + par
                        for ch in range(2):
                            qsl = slice(cq * 1024 + ch * 512,
                                        cq * 1024 + (ch + 1) * 512)
                            nc.tensor.matmul(
                                st[:, ch * 512:(ch + 1) * 512],
                                kF[hp, :, mt * 128:(mt + 1) * 128],
                                qF[hp, :, qsl],
                                perf_mode=mybir.MatmulPerfMode.DoubleRow,
                                tile_position=(32 * h, 0))
                        nc.scalar.activation(
                            e[:, par, cq * 1024:(cq + 1) * 1024], st[:],
                            AF.Exp, bias=zbias[:], scale=SCALE)

            for pt in range(4):
                qk_exp(pt)
            for nq in range(4):
                po = psPo.tile([65, 512], F32, tag="po", name=f"po{h}{nq}")
                sl = slice(nq * 512, (nq + 1) * 512)
                for pt in range(4):
                    nc.tensor.matmul(po[:],
                                     v_aug[:, 2 * pt:2 * pt + 2,
                                           65 * h:65 * h + 65],
                                     est[pt][:, :, sl],
                                     start=(pt == 0), stop=(pt == 3),
                                     perf_mode=mybir.MatmulPerfMode.DoubleRow)
                nc.vector.reciprocal(
                    rs_inv[:, h * NT + nq * 512:h * NT + (nq + 1) * 512],
                    po[64:65, :])
                nc.vector.tensor_copy(
                    oT[hr:hr + 64, cb, nq * 512:(nq + 1) * 512], po[0:64, :])

        for h in range(HEADS):
            attention_head(h)
            up_conv_block(0, h)
            up_conv_block(1, h)
            if h == 1:
                o_norm(0)
        o_norm(1)
        u_ln_stats()
        u_proj_merge([0, 1, 2, 3])


_NC_CACHE = {}


def get_nc(half=0):
    if half not in _NC_CACHE:
        _NC_CACHE[half] = build_nc(half)
    return _NC_CACHE[half]


def host_prep(q_w, q_b, kv_w, kv_b, sr_w, sr_b, sr_g, sr_be,
              up_w, up_b, upn_g, upn_be, proj_w, proj_b):
    f32 = np.float32
    w = {}
    # folded-parity channel permutation: partition P, parity s
    # -> channel 64*(P//32) + 2*(P%32) + s  (head P//32 in partitions 32h..)
    P = np.arange(128)
    cidx = 64 * (P // 32) + 2 * (P % 32)
    qT_w = q_w.T                                  # [cin, cout]
    qwtf = np.empty((128, 2, 2, 128), f32)
    kwt = kv_w[:C].T * sr_g[:, None]
    vwt = kv_w[C:].T * sr_g[:, None]
    kwtf = np.empty((128, 2, 2, 128), f32)
    for cb in range(2):
        for sp in range(2):
            qwtf[:, cb, sp, :] = qT_w[cb * 128:(cb + 1) * 128, cidx + sp]
            kwtf[:, cb, sp, :] = kwt[cb * 128:(cb + 1) * 128, cidx + sp]
    w["qwtf"] = qwtf
    w["kwtf"] = kwtf
    qbf = np.empty((128, 2), f32)
    kbv = kv_b[:C] + sr_be @ kwt
    kbf = np.empty((128, 2), f32)
    for sp in range(2):
        qbf[:, sp] = q_b[cidx + sp]
        kbf[:, sp] = kbv[cidx + sp]
    w["qbf"] = qbf
    w["kbf"] = kbf
    w["vwt"] = vwt
    vbe = kv_b[C:] + sr_be @ vwt
    w["vb"] = vbe.reshape(1, C)
    w["vbp"] = vbe.reshape(2, 128).T
    w["srb"] = sr_b.reshape(2, 128).T
    w["upb"] = up_b.reshape(C, 4).reshape(2, 128, 4).transpose(1, 0, 2)
    pwt = proj_w.T
    w["pwt"] = pwt
    w["pwtu"] = pwt * upn_g[:, None]
    w["pb"] = (proj_b + upn_be @ pwt).reshape(1, C)
    w["wcsn"] = (-w["pwtu"].sum(axis=0)).reshape(1, C)
    sel = np.zeros((2, 128), f32)
    sel[0, :64] = 1.0
    sel[1, 64:] = 1.0
    w["sel"] = sel

    # diagonal-expanded depthwise conv weights
    srw = sr_w.reshape(C, 9)                      # [c, t]
    srdgi = np.zeros((128, 2, 9, 128), f32)
    srdgi[np.arange(128), :, :, np.arange(128)] = \
        srw.reshape(2, 128, 9).transpose(1, 0, 2)
    w["srdgi"] = srdgi
    upw = up_w.reshape(C, 4, 9)                   # [c, pl, t]
    updgi = np.zeros((128, 2, 4, 9, 128), f32)
    updgi[np.arange(128), :, :, :, np.arange(128)] = \
        upw.reshape(2, 128, 4, 9).transpose(1, 0, 2, 3)
    w["updgi"] = updgi

    res = {}
    for k, v in w.items():
        dt = ml_dtypes.bfloat16 if k in BF_INPUTS else f32
        res[k] = np.ascontiguousarray(np.asarray(v, f32).astype(dt))
    return res


def make_in_maps(x, w):
    bf = ml_dtypes.bfloat16
    ximgs = []
    for b in range(B):
        xi = x[b].T.reshape(2, 128, H, W)          # [cb, p, 64, 64]
        pad = np.zeros((2, 128, H + 2, W + 2), np.float32)
        pad[:, :, 1:H + 1, 1:W + 1] = xi
        ximgs.append(np.ascontiguousarray(
            pad.transpose(1, 0, 2, 3).reshape(128, 2, 66 * 66).astype(bf)))
    in_maps = []
    for core in range(8):
        b, half = core // 2, core % 2
        m = dict(w)
        m["ximg"] = ximgs[b]
        xh = x[b, half * NT:(half + 1) * NT]       # [NT, C]
        m["xqT"] = np.ascontiguousarray(
            xh.T.reshape(2, 128, NT).transpose(1, 0, 2).astype(bf))
        in_maps.append(m)
    return in_maps


def kernel(x, q_w, q_b, kv_w, kv_b, sr_w, sr_b, sr_g, sr_be,
           up_w, up_b, upn_g, upn_be, proj_w, proj_b, H, W):
    assert int(H) == 64 and int(W) == 64
    f32 = np.float32
    x = np.asarray(x, f32)
    w = host_prep(np.asarray(q_w, f32), np.asarray(q_b, f32),
                  np.asarray(kv_w, f32), np.asarray(kv_b, f32),
                  np.asarray(sr_w, f32), np.asarray(sr_b, f32),
                  np.asarray(sr_g, f32), np.asarray(sr_be, f32),
                  np.asarray(up_w, f32), np.asarray(up_b, f32),
                  np.asarray(upn_g, f32), np.asarray(upn_be, f32),
                  np.asarray(proj_w, f32), np.asarray(proj_b, f32))
    in_maps = make_in_maps(x, w)
    out = np.empty((B, N, C), f32)
    for half in range(2):
        nc = get_nc(half)
        maps = [in_maps[2 * b + half] for b in range(B)]
        res = None
        for attempt in range(3):
            try:
                res = run_bass_kernel_spmd(nc, maps, core_ids=list(range(4))).results
                break
            except Exception:
                if attempt == 2:
                    raise
        assert res is not None
        for b in range(B):
            out[b, half * NT:(half + 1) * NT] = res[b]["out"]
    return out


# revision 49
# speedup vs baseline: 1.0309x; 1.0248x over previous
"""Trainium2 Bass kernel for nn_Attention_65180423685043 (sparse_attention).

Sharding: 8 cores = 4 batches x 2 sequence-halves. Each core computes the
spatial-reduction/kv branch for its batch (duplicated within the pair) and
attention + the up branch only for its 2048-token half. No collectives.

Host prep ships layout-transformed inputs (no arithmetic on x beyond dtype
cast): channel-major padded image ximg, transposed xqT, and diagonal-expanded
depthwise conv weights. Two compiled variants exist because the up-conv tap
row offset (16*half) is a compile-time AP constant.
"""
import numpy as np
import ml_dtypes

import concourse.bass as bass
import concourse.mybir as mybir
import concourse.tile as tile
from concourse import bacc
from concourse.bass_utils import run_bass_kernel_spmd
from concourse.masks import make_identity

F32 = mybir.dt.float32
BF16 = mybir.dt.bfloat16
F8 = mybir.dt.float8e4
AF = mybir.ActivationFunctionType
ALU = mybir.AluOpType

B, N, C, HEADS, SR = 4, 4096, 256, 4, 2
HD = C // HEADS
H = W = 64
Hs = Ws = 32
M = Hs * Ws          # kv tokens
NT = N // 2          # tokens per core
SCALE = HD ** -0.5
EPS = 1e-6

BF_INPUTS = {"ximg", "xqT", "srdgi", "updgi", "qwtf", "kwtf", "vwt",
             "pwt", "pwtu", "vb", "pb", "sel", "wcsn"}


def build_nc(half):
    nc = bacc.Bacc("TRN2", target_bir_lowering=False, debug=False, num_devices=8)
    d = {}

    def din(name, shape):
        dt = BF16 if name in BF_INPUTS else F32
        d[name] = nc.dram_tensor(name, list(shape), dt, kind="ExternalInput").ap()

    din("ximg", (128, 2, 66 * 66))
    din("xqT", (128, 2, NT))
    din("srdgi", (128, 2, 9, 128))
    din("updgi", (128, 2, 4, 9, 128))
    din("qwtf", (128, 2, 2, 128)); din("qbf", (128, 2))
    din("kwtf", (128, 2, 2, 128)); din("kbf", (128, 2))
    din("vwt", (C, C)); din("vb", (1, C)); din("vbp", (128, 2))
    din("srb", (128, 2))
    din("upb", (128, 2, 4))
    din("pwt", (C, C)); din("pwtu", (C, C)); din("pb", (1, C))
    din("wcsn", (1, C))
    din("sel", (2, 128))
    out = nc.dram_tensor("out", [NT, C], F32, kind="ExternalOutput").ap()

    with tile.TileContext(nc) as tc:
        with nc.allow_low_precision(reason="bf16 matmul operand pipeline"):
            body(nc, tc, d, out, half)
    nc.compile()
    return nc


def body(nc, tc, d, out, half):
    from contextlib import ExitStack
    ctx = ExitStack()
    with ctx:
        consts = ctx.enter_context(tc.tile_pool(name="consts", bufs=1))
        bigA = ctx.enter_context(tc.tile_pool(name="bigA", bufs=2))   # ximg / u
        bigB = ctx.enter_context(tc.tile_pool(name="bigB", bufs=1))   # xqT -> o_out
        persist = ctx.enter_context(tc.tile_pool(name="persist", bufs=1))
        diagp = ctx.enter_context(tc.tile_pool(name="diagp", bufs=1))
        expp = ctx.enter_context(tc.tile_pool(name="expp", bufs=9))
        smalls = ctx.enter_context(tc.tile_pool(name="smalls", bufs=2))
        sq = ctx.enter_context(tc.tile_pool(name="sq", bufs=2))
        outp = ctx.enter_context(tc.tile_pool(name="outp", bufs=2))
        psBig = ctx.enter_context(
            tc.tile_pool(name="psBig", bufs=2, space="PSUM"))   # [128,1024] f32
        psMid = ctx.enter_context(
            tc.tile_pool(name="psMid", bufs=2, space="PSUM"))   # [128,512] f32
        psPo = ctx.enter_context(
            tc.tile_pool(name="psPo", bufs=2, space="PSUM"))    # [65,512] f32

        # ---------- consts / inputs ----------
        def cload(name, shape, ap=None):
            dt = BF16 if name in BF_INPUTS else F32
            t = consts.tile(shape, dt, tag=name, name=f"c_{name}")
            nc.sync.dma_start(out=t[:], in_=ap if ap is not None else d[name])
            return t

        ones_col = consts.tile([128, 1], BF16, tag="ones_col")
        nc.gpsimd.memset(ones_col[:], 1.0)
        ones_row = consts.tile([1, 128], BF16, tag="ones_row")
        nc.gpsimd.memset(ones_row[:], 1.0)
        zbias = consts.tile([128, 1], F32, tag="zbias")
        nc.gpsimd.memset(zbias[:], 0.0)
        ebias = consts.tile([1, 1], F32, tag="ebias")
        nc.gpsimd.memset(ebias[:], EPS)
        zbias1 = consts.tile([1, 1], F32, tag="zbias1")
        nc.gpsimd.memset(zbias1[:], 0.0)
        ebias128 = consts.tile([128, 1], F32, tag="ebias128")
        nc.gpsimd.memset(ebias128[:], EPS)

        # ---------- persistent tiles ----------
        ximg = [bigA.tile([128, 66 * 66], BF16, tag="bigA", name=f"ximg{i}")
                for i in range(2)]
        xqT = bigB.tile([128, 2 * NT], BF16, tag="bigB")
        qF = persist.tile([128, 2, NT], F8, tag="qF")
        kF = persist.tile([128, 2, M], F8, tag="kF")
        xkvT = persist.tile([128, 2, M], BF16, tag="xkvT")
        v_aug = persist.tile([128, 8, 272], F8, tag="v_aug")
        vimg = persist.tile([128, 2, 34 * 34], BF16, tag="vimg")
        oT = persist.tile([128, 2, NT], BF16, tag="oT")
        rs_inv = persist.tile([1, 4 * NT], BF16, tag="rs_inv")
        ruT = persist.tile([128, 16], F32, tag="ruT")
        rm_rows = persist.tile([1, NT], BF16, tag="rm_rows")
        ident = consts.tile([128, 128], BF16, tag="ident")
        make_identity(nc, ident[:])

        # input DMAs, ordered so sr-conv (srdg+ximg) can start first, then the
        # LN/k chain (srb, kwt), then the q chain, then late-needed weights.
        srdg = diagp.tile([128, 2, 9, 128], BF16, tag="srdg")
        nc.sync.dma_start(out=srdg[:], in_=d["srdgi"])
        for cb in range(2):
            nc.sync.dma_start(out=ximg[cb][:], in_=d["ximg"][:, cb, :])
        srb = cload("srb", [128, 2])
        kwtf = cload("kwtf", [128, 2, 2, 128])
        vwt = cload("vwt", [128, 2, C], d["vwt"].rearrange("(a p) c -> p a c", a=2))
        qbf = cload("qbf", [128, 2]); kbf = cload("kbf", [128, 2])
        vb = cload("vb", [1, C]); vbp = cload("vbp", [128, 2])
        nc.sync.dma_start(out=xqT[:].rearrange("p (a c) -> p a c", a=2),
                          in_=d["xqT"])
        qwtf = cload("qwtf", [128, 2, 2, 128])
        sel0 = cload("sel", [1, 128], d["sel"][0:1, :])
        sel1 = consts.tile([1, 128], BF16, tag="sel1", name="c_sel1")
        nc.sync.dma_start(out=sel1[:], in_=d["sel"][1:2, :])
        pwt = cload("pwt", [128, 2, C], d["pwt"].rearrange("(a p) c -> p a c", a=2))
        pwtu = cload("pwtu", [128, 2, C], d["pwtu"].rearrange("(a p) c -> p a c", a=2))
        pb = cload("pb", [1, C])
        wcsn = cload("wcsn", [1, C])
        upb = cload("upb", [128, 2, 4])
        updg = diagp.tile([128, 2, 4, 9, 128], BF16, tag="updg")
        nc.sync.dma_start(out=updg[:], in_=d["updgi"])

        # vimg border zeroing (interior fully overwritten by vT-proj)
        for vo in range(2):
            vw = vimg[:, vo, :].rearrange("p (q c) -> p q c", q=34)
            nc.gpsimd.memset(vw[:, 0, :], 0.0)
            nc.gpsimd.memset(vw[:, 33, :], 0.0)
            nc.gpsimd.memset(vw[:, :, 0], 0.0)
            nc.gpsimd.memset(vw[:, :, 33], 0.0)
        ones_cols_view = bass.AP(
            tensor=v_aug.tensor, offset=v_aug.offset + 64,
            ap=[v_aug.ap[0], [272, 8], [65, 4], [1, 1]])
        nc.gpsimd.memset(ones_cols_view, 1.0)

        # ---------- sr depthwise conv (k=3, stride 2, pad 1) ----------
        x2s = {}
        for cb in range(2):
            pa = psBig.tile([128, M], F32, tag="big", name=f"pa{cb}")
            xv = ximg[cb][:].rearrange(
                "p (q a w b) -> p q a w b", q=33, a=2, b=2)
            for t in range(9):
                di, dj = t // 3, t % 3
                tap = xv[:, di // 2:di // 2 + 32, di % 2,
                         dj // 2:dj // 2 + 32, dj % 2]
                for ch in range(2):
                    nc.tensor.matmul(
                        pa[:, ch * 512:(ch + 1) * 512],
                        srdg[:, cb, t, :],
                        tap[:, ch * 16:(ch + 1) * 16, :],
                        start=(t == 0), stop=(t == 8))
            x2s[cb] = sq.tile([128, M], BF16, tag="squ", name=f"x2{cb}")
            for ch in range(2):
                sl = slice(ch * 512, (ch + 1) * 512)
                nc.scalar.activation(xkvT[:, cb, sl], pa[:, sl], AF.Identity,
                                     bias=srb[:, cb:cb + 1])
                nc.scalar.activation(x2s[cb][:, sl], pa[:, sl], AF.Square,
                                     bias=srb[:, cb:cb + 1])

        # ---------- LN over C for x_kv (g/be folded into kv weights) ----------
        for ch in range(2):
            sl = slice(ch * 512, (ch + 1) * 512)
            sx = psBig.tile([1, 512], F32, tag="big", name=f"sx{ch}")
            sx2 = psBig.tile([1, 512], F32, tag="big", name=f"sx2{ch}")
            for cb in range(2):
                nc.tensor.matmul(sx[:], ones_col[:], xkvT[:, cb, sl],
                                 start=(cb == 0), stop=(cb == 1))
                nc.tensor.matmul(sx2[:], ones_col[:], x2s[cb][:, sl],
                                 start=(cb == 0), stop=(cb == 1))
            mean = smalls.tile([1, 512], BF16, tag="mean", name=f"xmean{ch}", bufs=5)
            rstd = smalls.tile([1, 512], BF16, tag="rstd", name=f"xrstd{ch}", bufs=5)
            sA = smalls.tile([1, 512], F32, tag="sA", name=f"xsA{ch}")
            sB = smalls.tile([1, 512], F32, tag="sB", name=f"xsB{ch}")
            nc.vector.tensor_scalar_mul(mean[:], sx[:], 1.0 / C)
            nc.vector.tensor_mul(sB[:], mean[:], mean[:])
            nc.vector.scalar_tensor_tensor(sA[:], sx2[:], 1.0 / C, sB[:],
                                           op0=ALU.mult, op1=ALU.subtract)
            nc.scalar.activation(rstd[:], sA[:], AF.Abs_reciprocal_sqrt,
                                 bias=ebias[:])
            mb = psBig.tile([128, 512], F32, tag="big", name=f"mb{ch}")
            rb = psBig.tile([128, 512], F32, tag="big", name=f"rb{ch}")
            nc.tensor.matmul(mb[:], ones_row[:], mean[:])
            nc.tensor.matmul(rb[:], ones_row[:], rstd[:])
            for cb in range(2):
                nc.vector.tensor_sub(xkvT[:, cb, sl], xkvT[:, cb, sl], mb[:])
                nc.vector.tensor_mul(xkvT[:, cb, sl], xkvT[:, cb, sl], rb[:])

        # ---------- q projection (folded-parity planes, fp8 out) ----------
        for sp in range(2):
            for cq in range(2):
                pq = psBig.tile([128, 1024], F32, tag="big", name=f"pq{sp}{cq}")
                for cb in range(2):
                    for ch in range(2):
                        so = slice(ch * 512, (ch + 1) * 512)
                        nc.tensor.matmul(
                            pq[:, so], qwtf[:, cb, sp, :],
                            xqT[:, cb * NT + cq * 1024 + ch * 512:
                                cb * NT + cq * 1024 + (ch + 1) * 512],
                            start=(cb == 0), stop=(cb == 1))
                nc.scalar.activation(qF[:, sp, cq * 1024:(cq + 1) * 1024], pq[:],
                                     AF.Identity, bias=qbf[:, sp:sp + 1])

        # ---------- k / v projections ----------
        for sp in range(2):
            pk = psBig.tile([128, M], F32, tag="big", name=f"pk{sp}")
            for cb in range(2):
                for ch in range(2):
                    sl = slice(ch * 512, (ch + 1) * 512)
                    nc.tensor.matmul(
                        pk[:, sl], kwtf[:, cb, sp, :],
                        xkvT[:, cb, sl], start=(cb == 0), stop=(cb == 1))
            nc.scalar.activation(kF[:, sp, :], pk[:], AF.Identity,
                                 bias=kbf[:, sp:sp + 1])
        for mt in range(8):
            pv = psMid.tile([128, 256], F32, tag="mid", name=f"pv{mt}")
            for cb in range(2):
                nc.tensor.matmul(pv[:], xkvT[:, cb, mt * 128:(mt + 1) * 128],
                                 vwt[:, cb, :], start=(cb == 0), stop=False)
            nc.tensor.matmul(pv[:], ones_row[:], vb[:],
                             start=False, stop=True)
            va_dst = bass.AP(
                tensor=v_aug.tensor, offset=v_aug.offset + mt * 272,
                ap=[v_aug.ap[0], [65, 4], [1, 64]])
            nc.vector.tensor_copy(va_dst, pv[:].rearrange("p (a b) -> p a b", a=4))

        # ---------- vT projection -> vimg (channels-major v) ----------
        for vo in range(2):
            pvt = psBig.tile([128, M], F32, tag="big", name=f"pvt{vo}")
            for cb in range(2):
                for ch in range(2):
                    sl = slice(ch * 512, (ch + 1) * 512)
                    nc.tensor.matmul(
                        pvt[:, sl], vwt[:, cb, vo * 128:(vo + 1) * 128],
                        xkvT[:, cb, sl], start=(cb == 0), stop=(cb == 1))
            dst = vimg[:, vo, :].rearrange("p (q c) -> p q c", q=34)[
                :, 1:33, 1:33]
            nc.scalar.activation(
                dst, pvt[:].rearrange("p (a b) -> p a b", a=32),
                AF.Identity, bias=vbp[:, vo:vo + 1])

        # ---------- attention + interleaved up-branch (own half only) ----------
        ustats = {}
        vv = vimg[:].rearrange("p a (q w) -> p a q w", q=34)
        u = bigA.tile([128, 2, NT], BF16, tag="bigA", name="u")

        def up_conv_block(cb, pl):
            dg = updg[:, cb, pl]
            pu = psMid.tile([128, 512], F32, tag="mid", name=f"pu{cb}{pl}")
            for t in range(9):
                di, dj = t // 3, t % 3
                tap = vv[:, cb, di + 16 * half:di + 16 * half + 16, dj:dj + 32]
                nc.tensor.matmul(pu[:], dg[:, t, :], tap,
                                 start=(t == 0), stop=(t == 8))
            r1, r2 = pl // 2, pl % 2
            dst = u[:, cb, :].rearrange(
                "p (a x b y) -> p a x b y", a=16, x=2, y=2)[:, :, r1, :, r2]
            nc.vector.tensor_scalar_add(dst, pu[:].rearrange(
                "p (a b) -> p a b", a=16), upb[:, cb, pl:pl + 1])

        def u_ln_stats():
            # squares on Pool (idle in the tail); channel-sums directly in
            # token-partition layout via data-as-lhsT matmuls (out free = 1).
            u2s = []
            for cb in range(2):
                u2 = sq.tile([128, NT], BF16, tag="squ", name=f"u2{cb}")
                nc.vector.tensor_mul(u2[:], u[:, cb, :], u[:, cb, :])
                u2s.append(u2)
            # start=True resets the WHOLE psum bank, so pre-zero once and
            # accumulate everything with start=False
            su = psMid.tile([128, 2, 16], F32, tag="mid", name="suT")
            nc.vector.memset(su[:], 0.0)
            for ntl in range(16):
                tb = slice(ntl * 128, (ntl + 1) * 128)
                for cb in range(2):
                    nc.tensor.matmul(su[:, 0, ntl:ntl + 1], u[:, cb, tb],
                                     ones_col[:], start=False,
                                     stop=(cb == 1), skip_group_check=True)
                for cb in range(2):
                    nc.tensor.matmul(su[:, 1, ntl:ntl + 1], u2s[cb][:, tb],
                                     ones_col[:], start=False,
                                     stop=(cb == 1), skip_group_check=True)
            meanT = smalls.tile([128, 16], F32, tag="mean", name="umeanT", bufs=5)
            m2T = smalls.tile([128, 16], F32, tag="sB", name="um2T")
            varT = smalls.tile([128, 16], F32, tag="sA", name="uvarT")
            rmT = smalls.tile([128, 16], BF16, tag="rstd", name="urmT", bufs=5)
            nc.vector.tensor_scalar_mul(meanT[:], su[:, 0, :], 1.0 / C)
            nc.vector.tensor_mul(m2T[:], meanT[:], meanT[:])
            nc.vector.scalar_tensor_tensor(varT[:], su[:, 1, :], 1.0 / C,
                                           m2T[:], op0=ALU.mult,
                                           op1=ALU.subtract)
            nc.scalar.activation(ruT[:], varT[:], AF.Abs_reciprocal_sqrt,
                                 bias=ebias128[:])
            # merge applies ru to the whole psum, so the rank-1 correction
            # carries only the mean (ru*(u@W + m*wcsn) = r*(u@W) - r*m*colsum)
            nc.vector.tensor_copy(rmT[:], meanT[:])
            # PE-transpose rmT to [16,128], then per-row evac to a single
            # partition so rank-1 lhsT slices have base partition 0
            prm = psMid.tile([16, 128], BF16, tag="mid", name="prm")
            nc.tensor.transpose(prm[:], rmT[:], ident[:])
            rm16 = smalls.tile([16, 128], BF16, tag="rm16", name="rm16")
            nc.vector.tensor_copy(rm16[:], prm[:])
            nc.sync.dma_start(
                out=rm_rows[0:1, :].rearrange("a (p c) -> a p c", p=16),
                in_=rm16[:])

        def o_norm(cb):
            for cq in range(2):
                rbo = psBig.tile([128, 1024], F32, tag="big", name=f"rbo{cb}{cq}")
                for ch in range(2):
                    so = slice(ch * 512, (ch + 1) * 512)
                    h0, h1 = 2 * cb, 2 * cb + 1
                    base = cq * 1024 + ch * 512
                    nc.tensor.matmul(rbo[:, so], sel0[:],
                                     rs_inv[:, h0 * NT + base:h0 * NT + base + 512],
                                     start=True, stop=False)
                    nc.tensor.matmul(rbo[:, so], sel1[:],
                                     rs_inv[:, h1 * NT + base:h1 * NT + base + 512],
                                     start=False, stop=True)
                nc.vector.tensor_mul(oT[:, cb, cq * 1024:(cq + 1) * 1024],
                                     oT[:, cb, cq * 1024:(cq + 1) * 1024], rbo[:])

        def u_proj_merge(ngs):
            # fused: psum pp[:,0,:] = u@pwtu + rm⊗wcsn, pp[:,1,:] = oTn@pwt+pb
            # merge: ot = pp_u * ru[token] + pp_o   (per-token scalar on DVE)
            for ng in ngs:
                ot = outp.tile([128, 4, 256], F32, tag="outt", name=f"ot{ng}")
                for j in range(4):
                    ntl = ng * 4 + j
                    tb = slice(ntl * 128, (ntl + 1) * 128)
                    ppu = psMid.tile([128, 256], F32, tag="mid",
                                     name=f"ppu{ng}{j}")
                    ppo = psMid.tile([128, 256], F32, tag="mid",
                                     name=f"ppo{ng}{j}")
                    for cb in range(2):
                        nc.tensor.matmul(ppu[:], u[:, cb, tb],
                                         pwtu[:, cb, :],
                                         start=(cb == 0), stop=False)
                    nc.tensor.matmul(ppu[:],
                                     rm_rows[0:1, ntl * 128:(ntl + 1) * 128],
                                     wcsn[:], start=False, stop=True)
                    for cb in range(2):
                        nc.tensor.matmul(ppo[:], oT[:, cb, tb],
                                         pwt[:, cb, :],
                                         start=(cb == 0), stop=False)
                    nc.tensor.matmul(ppo[:], ones_row[:], pb[:],
                                     start=False, stop=True)
                    # ACT evac applies the per-token LN scale; DVE adds o-psum
                    # (walrus rejects two PSUM tensor operands in one DVE op)
                    usc = sq.tile([128, 256], BF16, tag="ppsb",
                                  name=f"usc{ng}{j}")
                    nc.scalar.activation(usc[:], ppu[:], AF.Identity,
                                         bias=zbias[:],
                                         scale=ruT[:, ntl:ntl + 1])
                    nc.vector.tensor_add(ot[:, j, :], usc[:], ppo[:])
                nt0 = ng * 4
                nc.sync.dma_start(
                    out=out[nt0 * 128:(nt0 + 4) * 128, :].rearrange(
                        "(a p) c -> p a c", a=4),
                    in_=ot[:])

        def qk_exp_phase(h):
            est = []
            hp = slice(32 * h, 32 * h + 32)
            for pt in range(4):
                e = expp.tile([128, 2, NT], F8, tag="expst", name=f"e{h}_{pt}")
                est.append(e)
                for par in range(2):
                    for cq in range(2):
                        st = psBig.tile([128, 1024], F32, tag="big",
                                        name=f"st{h}_{pt}_{par}_{cq}")
                        mt = pt * 2 + par
                        for ch in range(2):
                            qsl = slice(cq * 1024 + ch * 512,
                                        cq * 1024 + (ch + 1) * 512)
                            nc.tensor.matmul(
                                st[:, ch * 512:(ch + 1) * 512],
                                kF[hp, :, mt * 128:(mt + 1) * 128],
                                qF[hp, :, qsl],
                                perf_mode=mybir.MatmulPerfMode.DoubleRow,
                                tile_position=(32 * h, 0))
                        nc.scalar.activation(
                            e[:, par, cq * 1024:(cq + 1) * 1024], st[:],
                            AF.Exp, bias=zbias[:], scale=SCALE)
            return est

        def av_phase(h, est):
            cb, hr = h // 2, (h % 2) * 64
            for nq in range(4):
                po = psPo.tile([65, 512], F32, tag="po", name=f"po{h}{nq}")
                sl = slice(nq * 512, (nq + 1) * 512)
                for pt in range(4):
                    nc.tensor.matmul(po[:],
                                     v_aug[:, 2 * pt:2 * pt + 2,
                                           65 * h:65 * h + 65],
                                     est[pt][:, :, sl],
                                     start=(pt == 0), stop=(pt == 3),
                                     perf_mode=mybir.MatmulPerfMode.DoubleRow)
                nc.vector.reciprocal(
                    rs_inv[:, h * NT + nq * 512:h * NT + (nq + 1) * 512],
                    po[64:65, :])
                nc.vector.tensor_copy(
                    oT[hr:hr + 64, cb, nq * 512:(nq + 1) * 512], po[0:64, :])

        # software-pipelined: head h+1's QK/exp enters the PE queue before
        # head h's AV so the ACT exp stream never starves at head boundaries
        prev = None
        for h in range(HEADS):
            est = qk_exp_phase(h)
            if prev is not None:
                ph = h - 1
                av_phase(ph, prev)
                up_conv_block(0, ph)
                up_conv_block(1, ph)
                if ph == 1:
                    o_norm(0)
            prev = est
        av_phase(3, prev)
        up_conv_block(0, 3)
        up_conv_block(1, 3)
        o_norm(1)
        u_ln_stats()
        u_proj_merge([0, 1, 2, 3])


_NC_CACHE = {}


def get_nc(half=0):
    if half not in _NC_CACHE:
        _NC_CACHE[half] = build_nc(half)
    return _NC_CACHE[half]


def host_prep(q_w, q_b, kv_w, kv_b, sr_w, sr_b, sr_g, sr_be,
              up_w, up_b, upn_g, upn_be, proj_w, proj_b):
    f32 = np.float32
    w = {}
    # folded-parity channel permutation: partition P, parity s
    # -> channel 64*(P//32) + 2*(P%32) + s  (head P//32 in partitions 32h..)
    P = np.arange(128)
    cidx = 64 * (P // 32) + 2 * (P % 32)
    qT_w = q_w.T                                  # [cin, cout]
    qwtf = np.empty((128, 2, 2, 128), f32)
    kwt = kv_w[:C].T * sr_g[:, None]
    vwt = kv_w[C:].T * sr_g[:, None]
    kwtf = np.empty((128, 2, 2, 128), f32)
    for cb in range(2):
        for sp in range(2):
            qwtf[:, cb, sp, :] = qT_w[cb * 128:(cb + 1) * 128, cidx + sp]
            kwtf[:, cb, sp, :] = kwt[cb * 128:(cb + 1) * 128, cidx + sp]
    w["qwtf"] = qwtf
    w["kwtf"] = kwtf
    qbf = np.empty((128, 2), f32)
    kbv = kv_b[:C] + sr_be @ kwt
    kbf = np.empty((128, 2), f32)
    for sp in range(2):
        qbf[:, sp] = q_b[cidx + sp]
        kbf[:, sp] = kbv[cidx + sp]
    w["qbf"] = qbf
    w["kbf"] = kbf
    w["vwt"] = vwt
    vbe = kv_b[C:] + sr_be @ vwt
    w["vb"] = vbe.reshape(1, C)
    w["vbp"] = vbe.reshape(2, 128).T
    w["srb"] = sr_b.reshape(2, 128).T
    w["upb"] = up_b.reshape(C, 4).reshape(2, 128, 4).transpose(1, 0, 2)
    pwt = proj_w.T
    w["pwt"] = pwt
    w["pwtu"] = pwt * upn_g[:, None]
    w["pb"] = (proj_b + upn_be @ pwt).reshape(1, C)
    w["wcsn"] = (-w["pwtu"].sum(axis=0)).reshape(1, C)
    sel = np.zeros((2, 128), f32)
    sel[0, :64] = 1.0
    sel[1, 64:] = 1.0
    w["sel"] = sel

    # diagonal-expanded depthwise conv weights
    srw = sr_w.reshape(C, 9)                      # [c, t]
    srdgi = np.zeros((128, 2, 9, 128), f32)
    srdgi[np.arange(128), :, :, np.arange(128)] = \
        srw.reshape(2, 128, 9).transpose(1, 0, 2)
    w["srdgi"] = srdgi
    upw = up_w.reshape(C, 4, 9)                   # [c, pl, t]
    updgi = np.zeros((128, 2, 4, 9, 128), f32)
    updgi[np.arange(128), :, :, :, np.arange(128)] = \
        upw.reshape(2, 128, 4, 9).transpose(1, 0, 2, 3)
    w["updgi"] = updgi

    res = {}
    for k, v in w.items():
        dt = ml_dtypes.bfloat16 if k in BF_INPUTS else f32
        res[k] = np.ascontiguousarray(np.asarray(v, f32).astype(dt))
    return res


def make_in_maps(x, w):
    bf = ml_dtypes.bfloat16
    ximgs = []
    for b in range(B):
        xi = x[b].T.reshape(2, 128, H, W)          # [cb, p, 64, 64]
        pad = np.zeros((2, 128, H + 2, W + 2), np.float32)
        pad[:, :, 1:H + 1, 1:W + 1] = xi
        ximgs.append(np.ascontiguousarray(
            pad.transpose(1, 0, 2, 3).reshape(128, 2, 66 * 66).astype(bf)))
    in_maps = []
    for core in range(8):
        b, half = core // 2, core % 2
        m = dict(w)
        m["ximg"] = ximgs[b]
        xh = x[b, half * NT:(half + 1) * NT]       # [NT, C]
        m["xqT"] = np.ascontiguousarray(
            xh.T.reshape(2, 128, NT).transpose(1, 0, 2).astype(bf))
        in_maps.append(m)
    return in_maps


def kernel(x, q_w, q_b, kv_w, kv_b, sr_w, sr_b, sr_g, sr_be,
           up_w, up_b, upn_g, upn_be, proj_w, proj_b, H, W):
    assert int(H) == 64 and int(W) == 64
    f32 = np.float32
    x = np.asarray(x, f32)
    w = host_prep(np.asarray(q_w, f32), np.asarray(q_b, f32),
                  np.asarray(kv_w, f32), np.asarray(kv_b, f32),
                  np.asarray(sr_w, f32), np.asarray(sr_b, f32),
                  np.asarray(sr_g, f32), np.asarray(sr_be, f32),
                  np.asarray(up_w, f32), np.asarray(up_b, f32),
                  np.asarray(upn_g, f32), np.asarray(upn_be, f32),
                  np.asarray(proj_w, f32), np.asarray(proj_b, f32))
    in_maps = make_in_maps(x, w)
    out = np.empty((B, N, C), f32)
    for half in range(2):
        nc = get_nc(half)
        maps = [in_maps[2 * b + half] for b in range(B)]
        res = None
        for attempt in range(3):
            try:
                res = run_bass_kernel_spmd(nc, maps, core_ids=list(range(4))).results
                break
            except Exception:
                if attempt == 2:
                    raise
        assert res is not None
        for b in range(B):
            out[b, half * NT:(half + 1) * NT] = res[b]["out"]
    return out
